# revision 14
# baseline (speedup 1.0000x reference)
"""Trainium2 Bass kernel for BinaryDiffCol:

    y = x @ base + (x @ sign(mask)) * coeff

Since coeff scales output columns, the two GEMMs fold into ONE:

    y = x @ W,   W = base + sign * coeff   (sign in {-1,+1} unpacked from mask bits)

Column-parallel over 8 NeuronCores: core i handles output columns
[i*512, (i+1)*512). x is replicated; base/mask/coeff are column-sharded.

The contraction index is free to be enumerated in any order as long as x^T
rows and W rows agree. We use k' = j*128 + g (j = bit index, g = mask row),
so W k-tile j covers all 128 mask rows at one bit position:
  - mask loads ONCE (two uint16 halves), no 32x replication
  - bit extraction is a constant-shift dual-op (DVE fast modes apply)
  - sign*coeff is a sign-bit XOR against +/-coeff
x^T and base are host-permuted into k' order (pure relayout, same class as
the shard slicing itself; x is replicated either way).

Per-core device program (Tile framework):
  Phase A: build W[k', n] = base_perm + (coeff XOR signbit(maskbit)) in SBUF
  Phase B: single 4096x4096x512 bf16 GEMM; x^T k'-tiles stationary,
           W tiles moving; PSUM fp32 accumulation, bf16 output.
"""
import numpy as np
import ml_dtypes

import concourse.bass as bass
import concourse.tile as tile
from concourse import bacc, mybir
from concourse.bass_utils import run_bass_kernel_spmd

T = 4096          # tokens (rows of x / y)
K = 4096          # contraction dim
N = 4096          # total output columns
NCORES = 8
NS = N // NCORES  # 512 output columns per core
P = 128
KT = K // P       # 32 k-tiles (= bit index j in permuted order)
TSUP = 512        # rows per super-tile (4 PSUM banks)
NSUP = T // TSUP  # 8 super-tiles
SUBS = TSUP // P  # 4 psum tiles per super-tile

BF16 = mybir.dt.bfloat16
U16 = mybir.dt.uint16
I32 = mybir.dt.int32
F32 = mybir.dt.float32

_nc_cache = None


def _build():
    global _nc_cache
    if _nc_cache is not None:
        return _nc_cache

    nc = bacc.Bacc("TRN2", target_bir_lowering=False, debug=False)

    xt_d = nc.dram_tensor("xt", [K, T], BF16, kind="ExternalInput")
    base_d = nc.dram_tensor("base", [K, NS], BF16, kind="ExternalInput")
    coeffb_d = nc.dram_tensor("coeffb", [P, NS], BF16, kind="ExternalInput")
    mlo_d = nc.dram_tensor("mlo", [P, NS], U16, kind="ExternalInput")
    mhi_d = nc.dram_tensor("mhi", [P, NS], U16, kind="ExternalInput")
    y_d = nc.dram_tensor("y", [T, NS], BF16, kind="ExternalOutput")

    with tile.TileContext(nc) as tc:
        with (
            tc.tile_pool(name="wpool", bufs=1) as wpool,
            tc.tile_pool(name="consts", bufs=1) as consts,
            tc.tile_pool(name="bload", bufs=4) as bload,
            tc.tile_pool(name="wtmp", bufs=4) as wtmp,
            tc.tile_pool(name="xtp", bufs=8) as xtp,
            tc.tile_pool(name="outp", bufs=4) as outp,
            tc.tile_pool(name="psum", bufs=8, space="PSUM") as psum,
        ):
            dmac = [0]

            def hwdge():
                dmac[0] += 1
                return nc.sync if dmac[0] % 2 == 0 else nc.scalar

            # ---- Phase A: W[j-tile] = base_perm[j-tile] + sign(bit)*c ----
            # sign = 2*bit - 1:  W = base - (c XOR (bit<<15))
            #   bit=1 -> c sign-flipped -> base + c;  bit=0 -> base - c
            mlo_t = consts.tile([P, NS], U16)
            nc.sync.dma_start(mlo_t[:], mlo_d.ap())
            cf_t = consts.tile([P, NS], BF16)
            nc.scalar.dma_start(cf_t[:], coeffb_d.ap())
            mhi_t = consts.tile([P, NS], U16)
            nc.sync.dma_start(mhi_t[:], mhi_d.ap())

            w_all = wpool.tile([P, KT, NS], BF16)  # 4 MiB resident
            for j in range(KT):
                b_t = bload.tile([P, NS], BF16, tag="b")
                hwdge().dma_start(b_t[:], base_d.ap()[j * P:(j + 1) * P, :])

                src_t = mlo_t if j < 16 else mhi_t
                sh = j % 16
                # t = (src << (15-sh)) & 0x8000  -> {0, 0x8000}
                bit_t = wtmp.tile([P, NS], U16, tag="bit")
                nc.any.tensor_scalar(
                    bit_t[:], src_t[:], 15 - sh, 0x8000,
                    mybir.AluOpType.logical_shift_left,
                    mybir.AluOpType.bitwise_and,
                )
                sc_t = wtmp.tile([P, NS], U16, tag="sc")
                nc.any.tensor_tensor(
                    sc_t[:], bit_t[:], cf_t[:].bitcast(U16),
                    mybir.AluOpType.bitwise_xor,
                )
                nc.any.tensor_tensor(
                    w_all[:, j, :], b_t[:], sc_t[:].bitcast(BF16),
                    mybir.AluOpType.subtract,
                )

            # ---- Phase B: y = x @ W ----
            # First two super-tiles interleaved (halves early W demand rate
            # while W is still being built), then one at a time.
            def do_group(sups):
                accs = {
                    s: [psum.tile([P, NS], F32, tag="acc", name=f"acc{s}_{i}")
                        for i in range(SUBS)]
                    for s in sups
                }
                for kt in range(KT):
                    for s in sups:
                        xt_t = xtp.tile([P, TSUP], BF16, tag="xt",
                                        name=f"xt{s}_{kt}")
                        hwdge().dma_start(
                            xt_t[:],
                            xt_d.ap()[kt * P:(kt + 1) * P,
                                      s * TSUP:(s + 1) * TSUP],
                        )
                        for sub in range(SUBS):
                            nc.tensor.matmul(
                                accs[s][sub][:],
                                xt_t[:, sub * P:(sub + 1) * P],
                                w_all[:, kt, :],
                                start=(kt == 0),
                                stop=(kt == KT - 1),
                            )
                for s in sups:
                    for sub in range(SUBS):
                        o_t = outp.tile([P, NS], BF16, tag="o",
                                        name=f"o{s}_{sub}")
                        nc.any.tensor_copy(out=o_t[:], in_=accs[s][sub][:])
                        r0 = (s * SUBS + sub) * P
                        hwdge().dma_start(y_d.ap()[r0:r0 + P, :], o_t[:])

            do_group([0, 1])
            for s in range(2, NSUP):
                do_group([s])

    nc.compile()
    _nc_cache = nc
    return nc


def _prep_in_maps(x, base, coeff, mask):
    x = np.asarray(x, dtype=ml_dtypes.bfloat16)
    base = np.asarray(base, dtype=ml_dtypes.bfloat16)
    coeff = np.asarray(coeff, dtype=ml_dtypes.bfloat16)
    mask = np.asarray(mask, dtype=np.int32)

    # x^T in permuted k' = j*128 + g order:
    # xt_perm[j*128+g, t] = x[t, g*32+j]
    xt_perm = np.ascontiguousarray(
        x.reshape(T, P, 32).transpose(2, 1, 0).reshape(K, T))

    in_maps = []
    for c in range(NCORES):
        sl = slice(c * NS, (c + 1) * NS)
        base_sh = base[:, sl]
        base_perm = np.ascontiguousarray(
            base_sh.reshape(P, 32, NS).transpose(1, 0, 2).reshape(K, NS))
        m16 = mask[:, sl].view('<u2').reshape(P, NS, 2)
        in_maps.append({
            "xt": xt_perm,
            "base": base_perm,
            "coeffb": np.ascontiguousarray(
                np.broadcast_to(coeff[sl][None, :], (P, NS))),
            "mlo": np.ascontiguousarray(m16[:, :, 0]),
            "mhi": np.ascontiguousarray(m16[:, :, 1]),
        })
    return in_maps


def _run(x, base, coeff, mask, trace=False, **kw):
    nc = _build()
    in_maps = _prep_in_maps(x, base, coeff, mask)
    res = run_bass_kernel_spmd(nc, in_maps, list(range(NCORES)), trace=trace, **kw)
    y = np.concatenate([r["y"] for r in res.results], axis=1)
    return y, res


def kernel(x, base, coeff, mask):
    y, _ = _run(x, base, coeff, mask)
    return y


# revision 16
# speedup vs baseline: 1.0060x; 1.0060x over previous
"""Trainium2 Bass kernel for BinaryDiffCol:

    y = x @ base + (x @ sign(mask)) * coeff

Since coeff scales output columns, the two GEMMs fold into ONE:

    y = x @ W,   W = base + sign * coeff   (sign in {-1,+1} unpacked from mask bits)

Column-parallel over 8 NeuronCores: core i handles output columns
[i*512, (i+1)*512). x is replicated; base/mask/coeff are column-sharded.

The contraction index is free to be enumerated in any order as long as x^T
rows and W rows agree. We use k' = j*128 + g (j = bit index, g = mask row),
so W k-tile j covers all 128 mask rows at one bit position:
  - mask loads ONCE (two uint16 halves), no 32x replication
  - bit extraction is a constant-shift dual-op (DVE fast modes apply)
  - sign*coeff is a sign-bit XOR against +/-coeff
x^T and base are host-permuted into k' order (pure relayout, same class as
the shard slicing itself; x is replicated either way).

Per-core device program (Tile framework):
  Phase A: build W[k', n] = base_perm + (coeff XOR signbit(maskbit)) in SBUF
  Phase B: single 4096x4096x512 bf16 GEMM; x^T k'-tiles stationary,
           W tiles moving; PSUM fp32 accumulation, bf16 output.
"""
import numpy as np
import ml_dtypes

import concourse.bass as bass
import concourse.tile as tile
from concourse import bacc, mybir
from concourse.bass_utils import run_bass_kernel_spmd

T = 4096          # tokens (rows of x / y)
K = 4096          # contraction dim
N = 4096          # total output columns
NCORES = 8
NS = N // NCORES  # 512 output columns per core
P = 128
KT = K // P       # 32 k-tiles (= bit index j in permuted order)
TSUP = 512        # rows per super-tile (4 PSUM banks)
NSUP = T // TSUP  # 8 super-tiles
SUBS = TSUP // P  # 4 psum tiles per super-tile

BF16 = mybir.dt.bfloat16
U16 = mybir.dt.uint16
I32 = mybir.dt.int32
F32 = mybir.dt.float32

_nc_cache = None


def _build():
    global _nc_cache
    if _nc_cache is not None:
        return _nc_cache

    nc = bacc.Bacc("TRN2", target_bir_lowering=False, debug=False)

    xt_d = nc.dram_tensor("xt", [K, T], BF16, kind="ExternalInput")
    base_d = nc.dram_tensor("base", [K, NS], BF16, kind="ExternalInput")
    coeffb_d = nc.dram_tensor("coeffb", [P, NS], BF16, kind="ExternalInput")
    mlo_d = nc.dram_tensor("mlo", [P, NS], U16, kind="ExternalInput")
    mhi_d = nc.dram_tensor("mhi", [P, NS], U16, kind="ExternalInput")
    y_d = nc.dram_tensor("y", [T, NS], BF16, kind="ExternalOutput")

    with tile.TileContext(nc) as tc:
        with (
            tc.tile_pool(name="wpool", bufs=1) as wpool,
            tc.tile_pool(name="consts", bufs=1) as consts,
            tc.tile_pool(name="bload", bufs=4) as bload,
            tc.tile_pool(name="wtmp", bufs=4) as wtmp,
            tc.tile_pool(name="xtp", bufs=8) as xtp,
            tc.tile_pool(name="outp", bufs=4) as outp,
            tc.tile_pool(name="psum", bufs=8, space="PSUM") as psum,
        ):
            dmac = [0]

            def hwdge():
                dmac[0] += 1
                return nc.sync if dmac[0] % 2 == 0 else nc.scalar

            # ---- Phase A: W[j-tile] = base_perm[j-tile] + sign(bit)*c ----
            # sign = 2*bit - 1:  W = base - (c XOR (bit<<15))
            #   bit=1 -> c sign-flipped -> base + c;  bit=0 -> base - c
            mlo_t = consts.tile([P, NS], U16)
            nc.sync.dma_start(mlo_t[:], mlo_d.ap())
            cf_t = consts.tile([P, NS], BF16)
            nc.scalar.dma_start(cf_t[:], coeffb_d.ap())
            mhi_t = consts.tile([P, NS], U16)
            nc.sync.dma_start(mhi_t[:], mhi_d.ap())

            w_all = wpool.tile([P, KT, NS], BF16)  # 4 MiB resident
            b_ts = {}
            for j in range(KT):
                if j % 2 == 0:
                    # one 256 KB DMA covers two base j-tiles
                    b2_t = bload.tile([P, 2, NS], BF16, tag="b", name=f"b{j}")
                    hwdge().dma_start(
                        b2_t[:],
                        base_d.ap()[j * P:(j + 2) * P, :]
                        .rearrange("(a p) n -> p a n", p=P),
                    )
                    b_ts[j], b_ts[j + 1] = b2_t[:, 0, :], b2_t[:, 1, :]

                src_t = mlo_t if j < 16 else mhi_t
                sh = j % 16
                # t = (src << (15-sh)) & 0x8000  -> {0, 0x8000}
                bit_t = wtmp.tile([P, NS], U16, tag="bit")
                nc.any.tensor_scalar(
                    bit_t[:], src_t[:], 15 - sh, 0x8000,
                    mybir.AluOpType.logical_shift_left,
                    mybir.AluOpType.bitwise_and,
                )
                sc_t = wtmp.tile([P, NS], U16, tag="sc")
                nc.any.tensor_tensor(
                    sc_t[:], bit_t[:], cf_t[:].bitcast(U16),
                    mybir.AluOpType.bitwise_xor,
                )
                nc.any.tensor_tensor(
                    w_all[:, j, :], b_ts[j], sc_t[:].bitcast(BF16),
                    mybir.AluOpType.subtract,
                )

            # ---- Phase B: y = x @ W ----
            # First two super-tiles interleaved (halves early W demand rate
            # while W is still being built), then one at a time.
            def store_outputs(accs, sups):
                for s in sups:
                    for sub in range(0, SUBS, 2):
                        o_t = outp.tile([P, 2, NS], BF16, tag="o",
                                        name=f"o{s}_{sub}")
                        nc.any.tensor_copy(out=o_t[:, 0, :],
                                           in_=accs[s][sub][:])
                        nc.any.tensor_copy(out=o_t[:, 1, :],
                                           in_=accs[s][sub + 1][:])
                        r0 = (s * SUBS + sub) * P
                        hwdge().dma_start(
                            y_d.ap()[r0:r0 + 2 * P, :]
                            .rearrange("(a p) n -> p a n", p=P),
                            o_t[:],
                        )

            def mk_accs(sups):
                return {
                    s: [psum.tile([P, NS], F32, tag="acc", name=f"acc{s}_{i}")
                        for i in range(SUBS)]
                    for s in sups
                }

            # group 0: sups 0+1; one contiguous [128, 1024] xt load per kt
            accs = mk_accs([0, 1])
            for kt in range(KT):
                xtw = xtp.tile([P, 2 * TSUP], BF16, tag="xt", name=f"xtg{kt}")
                hwdge().dma_start(
                    xtw[:], xt_d.ap()[kt * P:(kt + 1) * P, 0:2 * TSUP])
                for s in (0, 1):
                    for sub in range(SUBS):
                        nc.tensor.matmul(
                            accs[s][sub][:],
                            xtw[:, s * TSUP + sub * P: s * TSUP + (sub + 1) * P],
                            w_all[:, kt, :],
                            start=(kt == 0),
                            stop=(kt == KT - 1),
                        )
            store_outputs(accs, [0, 1])

            # remaining sups: pair-merged xt loads (two k-tiles per DMA)
            for s in range(2, NSUP):
                accs = mk_accs([s])
                for kt2 in range(0, KT, 2):
                    xt2 = xtp.tile([P, 2, TSUP], BF16, tag="xt",
                                   name=f"xt{s}_{kt2}")
                    hwdge().dma_start(
                        xt2[:],
                        xt_d.ap()[kt2 * P:(kt2 + 2) * P,
                                  s * TSUP:(s + 1) * TSUP]
                        .rearrange("(a p) t -> p a t", p=P),
                    )
                    for a in (0, 1):
                        kt = kt2 + a
                        for sub in range(SUBS):
                            nc.tensor.matmul(
                                accs[s][sub][:],
                                xt2[:, a, sub * P:(sub + 1) * P],
                                w_all[:, kt, :],
                                start=(kt == 0),
                                stop=(kt == KT - 1),
                            )
                store_outputs(accs, [s])

    nc.compile()
    _nc_cache = nc
    return nc


def _prep_in_maps(x, base, coeff, mask):
    x = np.asarray(x, dtype=ml_dtypes.bfloat16)
    base = np.asarray(base, dtype=ml_dtypes.bfloat16)
    coeff = np.asarray(coeff, dtype=ml_dtypes.bfloat16)
    mask = np.asarray(mask, dtype=np.int32)

    # x^T in permuted k' = j*128 + g order:
    # xt_perm[j*128+g, t] = x[t, g*32+j]
    xt_perm = np.ascontiguousarray(
        x.reshape(T, P, 32).transpose(2, 1, 0).reshape(K, T))

    in_maps = []
    for c in range(NCORES):
        sl = slice(c * NS, (c + 1) * NS)
        base_sh = base[:, sl]
        base_perm = np.ascontiguousarray(
            base_sh.reshape(P, 32, NS).transpose(1, 0, 2).reshape(K, NS))
        m16 = mask[:, sl].view('<u2').reshape(P, NS, 2)
        in_maps.append({
            "xt": xt_perm,
            "base": base_perm,
            "coeffb": np.ascontiguousarray(
                np.broadcast_to(coeff[sl][None, :], (P, NS))),
            "mlo": np.ascontiguousarray(m16[:, :, 0]),
            "mhi": np.ascontiguousarray(m16[:, :, 1]),
        })
    return in_maps


def _run(x, base, coeff, mask, trace=False, **kw):
    nc = _build()
    in_maps = _prep_in_maps(x, base, coeff, mask)
    res = run_bass_kernel_spmd(nc, in_maps, list(range(NCORES)), trace=trace, **kw)
    y = np.concatenate([r["y"] for r in res.results], axis=1)
    return y, res


def kernel(x, base, coeff, mask):
    y, _ = _run(x, base, coeff, mask)
    return y
